# revision 8
# baseline (speedup 1.0000x reference)
"""Sliding-window local attention (KeOps ranges) on 8 Trainium2 cores.

Problem: B=4 H=16 T=4096 D=64, query block w=128 attends keys
[128(i-1), 128(i+1)) clamped to [0, T).  Softmax over the 256-key window,
out = attn @ V.  Only block 0 has out-of-range keys (its lower half), so
masking reduces to skipping that half-block.

Sharding: batch*head (64 pairs) split across 8 cores, 8 heads per core.

Per-core kernel (all matmuls bf16, fp32 PSUM accumulation):
  - Host pre-arranges all tensors into SBUF-native layouts so every DMA
    is a contiguous line-rate transfer:
      qt/kt: [128, T] per head pair (two heads' [64, T] Q^T/K^T stacked)
      vo:    [128, nblk*(D+1)] per head = exp(m) * [V | 1] blocks
             (per-key mask folded in; the ones column gives the softmax
             denominator through the AV matmul)
      o:     [128, nblk*D] per head, un-permuted on the host afterwards
  - S^T[k, q] = K_blk @ Q_blk^T via matmul(lhsT=K^T slice, rhs=Q^T slice),
    d=64 contraction on partitions [0:64] (head A) / [64:128] (head B).
    Key-major: one LDWEIGHTS per key block, N=256 moving operand covering
    the two query blocks that attend to it.  Groups of GRP=4 query blocks
    share one 2-bank PSUM tile [128, 8, 128]; the slot permutation keeps
    every N=256 matmul write inside a single bank.
  - One exp per group: ACT [128, 1024] PSUM -> SBUF bf16, scale=1/sqrt(d).
    (ACT costs (N+352)/1.2ns per op: batching is the whole game.)
    No max-subtraction: scores are O(+-5), fp32 exp is well-conditioned.
  - AV: matmul(lhsT=E^T slot, rhs=vo block) accumulating the two window
    halves into [128, GRP, 65] PSUM.
  - Normalize on DVE: one reciprocal [128, GRP] + one broadcasted
    tensor_mul per group into the fp32 output staging tile.
"""

import numpy as np
import ml_dtypes
from contextlib import ExitStack

import concourse.mybir as mybir
import concourse.tile as tile
from concourse import bacc
from concourse.bass_utils import run_bass_kernel_spmd

B, H, T, D = 4, 16, 4096, 64
W = 128                       # query/key block width
NCORES = 8
HPC = (B * H) // NCORES       # heads per core = 8
NPAIR = HPC // 2              # head pairs per core = 4
GRP = 4                       # query blocks per exp/normalize group
NBLK = T // W
BF16 = mybir.dt.bfloat16
FP32 = mybir.dt.float32

# Slot permutation inside one group's [128, 8, 128] PSUM tile.  Logical
# half-blocks, key-major: key block g0-1+x pairs with query blocks
# (g0-1+x, g0+x).  Slots are arranged so the three N=256 writes (key
# blocks g0, g0+1, g0+2) each live inside one 2KB PSUM bank:
#   bank A slots [0,1,2,3]: (kg0-1,qg0) (kg0,qg0) (kg0,qg0+1) (kg0+3,qg0+3)
#   bank B slots [4,5,6,7]: (kg0+1,qg0+1) (kg0+1,qg0+2) (kg0+2,qg0+2) (kg0+2,qg0+3)
# AV lookup: SLOT[bi][hi] = slot of (query block g0+bi, half hi) where
# half hi=0 is key block g0+bi-1, hi=1 is key block g0+bi.
SLOT = [(0, 1), (2, 4), (5, 6), (7, 3)]
# Score matmuls: (key offset dk from g0, first slot, n_query_blocks)
SMM = [(-1, 0, 1), (0, 1, 2), (1, 4, 2), (2, 6, 2), (3, 3, 1)]


def build_nc(t=T, npair=NPAIR, grp=GRP):
    """Build the single-core Bass program (SPMD across 8 cores)."""
    nblk = t // W
    ngrp = nblk // grp
    hpc = npair * 2
    nc = bacc.Bacc("TRN2", debug=False, enable_asserts=False)
    qtd = nc.dram_tensor("qt", [npair * W, t], BF16, kind="ExternalInput").ap()
    ktd = nc.dram_tensor("kt", [npair * W, t], BF16, kind="ExternalInput").ap()
    vod = nc.dram_tensor("vo", [hpc * W, nblk * (D + 1)], BF16,
                         kind="ExternalInput").ap()
    ood = nc.dram_tensor("o", [hpc * W, nblk * D], FP32, kind="ExternalOutput").ap()

    Exp = mybir.ActivationFunctionType.Exp
    with tile.TileContext(nc) as tc, ExitStack() as ctx:
        qk = ctx.enter_context(tc.tile_pool(name="qk", bufs=2))
        vp = ctx.enter_context(tc.tile_pool(name="vp", bufs=2))
        ep = ctx.enter_context(tc.tile_pool(name="ep", bufs=3))
        rp = ctx.enter_context(tc.tile_pool(name="rp", bufs=4))
        osp = ctx.enter_context(tc.tile_pool(name="osp", bufs=2))
        stp = ctx.enter_context(tc.tile_pool(name="stp", bufs=3, space="PSUM"))
        avp = ctx.enter_context(tc.tile_pool(name="avp", bufs=2, space="PSUM"))

        for pr in range(npair):
            qt = qk.tile([W, t], BF16, tag="qt")
            nc.sync.dma_start(out=qt[:], in_=qtd[pr * W:(pr + 1) * W, :])
            kt = qk.tile([W, t], BF16, tag="kt")
            nc.sync.dma_start(out=kt[:], in_=ktd[pr * W:(pr + 1) * W, :])
            vts, osts = [], []
            for e in range(2):
                h = 2 * pr + e
                vt = vp.tile([W, nblk, D + 1], BF16, tag=f"v{e}", name=f"vt{e}")
                nc.sync.dma_start(
                    out=vt[:],
                    in_=vod[h * W:(h + 1) * W, :].rearrange(
                        "p (n d) -> p n d", d=D + 1),
                )
                vts.append(vt)
                osts.append(osp.tile([W, nblk, D], FP32, tag=f"os{e}", name=f"ost{e}"))

            for g in range(ngrp):
                g0 = g * grp
                sts, ets_, avs = [], [], []
                for e in range(2):
                    dsl = slice(D * e, D * (e + 1))
                    st = stp.tile([W, 2 * grp, W], FP32, tag="st", name="st")
                    if g == 0:
                        # slot (block 0, half j=-1) never written; keep exp finite
                        nc.vector.memset(st[:, 0, :], 0.0)
                    for dk, s0, nq in SMM:
                        j = g0 + dk            # key block
                        if j < 0:
                            continue
                        qb0 = g0 if dk == -1 else j  # first query block covered
                        nc.tensor.matmul(
                            st[:, s0:s0 + nq, :],
                            kt[dsl, W * j:W * (j + 1)],
                            qt[dsl, W * qb0:W * (qb0 + nq)],
                            start=True, stop=True,
                        )
                    sts.append(st)
                for e in range(2):
                    et = ep.tile([W, 2 * grp, W], BF16, tag="et", name="et")
                    nc.scalar.activation(et[:], sts[e][:], Exp, scale=0.125)
                    ets_.append(et)
                for e in range(2):
                    av = avp.tile([W, grp, D + 1], FP32, tag="av", name="av")
                    for bi in range(grp):
                        i = g0 + bi
                        mms = [(SLOT[bi][hi], j)
                               for hi, j in enumerate((i - 1, i)) if j >= 0]
                        for x, (s, j) in enumerate(mms):
                            nc.tensor.matmul(
                                av[:, bi, :],
                                ets_[e][:, s, :], vts[e][:, j, :],
                                start=(x == 0), stop=(x == len(mms) - 1),
                            )
                    avs.append(av)
                for e in range(2):
                    rt = rp.tile([W, grp], FP32, tag="rt", name="rt")
                    nc.vector.reciprocal(rt[:], avs[e][:, :, D])
                    nc.vector.tensor_tensor(
                        out=osts[e][:, g0:g0 + grp, :],
                        in0=avs[e][:, :, 0:D],
                        in1=rt.rearrange("p (n o) -> p n o", o=1)
                            .broadcast_to([W, grp, D]),
                        op=mybir.AluOpType.mult,
                    )

            for e in range(2):
                h = 2 * pr + e
                nc.sync.dma_start(
                    out=ood[h * W:(h + 1) * W, :].rearrange(
                        "p (n d) -> p n d", d=D),
                    in_=osts[e][:],
                )
    nc.compile()
    return nc


_NC = None


def _get_nc():
    global _NC
    if _NC is None:
        _NC = build_nc()
    return _NC


def make_in_maps(query_layer, key_layer, value_layer, attention_mask):
    q = np.asarray(query_layer)
    k = np.asarray(key_layer)
    v = np.asarray(value_layer)
    m = np.asarray(attention_mask, dtype=np.float32)
    bf = ml_dtypes.bfloat16
    # [B*H, T, D] -> per-pair Q^T/K^T: [NPAIR_total, 2, D, T]
    qf = q.reshape(B * H, T, D)
    kf = k.reshape(B * H, T, D)
    em = np.exp(m)                                   # [B, T] per-key mask factor
    in_maps = []
    for c in range(NCORES):
        sl = slice(c * HPC, (c + 1) * HPC)
        b = (c * HPC) // H
        qc = (qf[sl].astype(bf).reshape(NPAIR, 2, T, D)
              .transpose(0, 1, 3, 2).reshape(NPAIR * W, T))
        kc = (kf[sl].astype(bf).reshape(NPAIR, 2, T, D)
              .transpose(0, 1, 3, 2).reshape(NPAIR * W, T))
        vc = np.empty((HPC, T, D + 1), np.float32)
        vc[:, :, :D] = v.reshape(B * H, T, D)[sl] * em[b][None, :, None]
        vc[:, :, D] = em[b][None, :]
        voc = (vc.astype(bf).reshape(HPC, NBLK, W, D + 1)
               .transpose(0, 2, 1, 3).reshape(HPC * W, NBLK * (D + 1)))
        in_maps.append({
            "qt": np.ascontiguousarray(qc),
            "kt": np.ascontiguousarray(kc),
            "vo": np.ascontiguousarray(voc),
        })
    return in_maps


def run(inputs, trace=False):
    """Run on the 8 cores; returns (full_output, BassKernelResults)."""
    in_maps = make_in_maps(**inputs)
    nc = _get_nc()
    res = run_bass_kernel_spmd(
        nc, in_maps, core_ids=list(range(NCORES)), trace=trace
    )
    out = np.empty((B * H, T, D), np.float32)
    for c in range(NCORES):
        oc = res.results[c]["o"].reshape(HPC, W, NBLK, D)
        out[c * HPC:(c + 1) * HPC] = (
            oc.transpose(0, 2, 1, 3).reshape(HPC, T, D))
    return out.reshape(B, H, T, D), res


def kernel(query_layer, key_layer, value_layer, attention_mask):
    out, _ = run({
        "query_layer": query_layer,
        "key_layer": key_layer,
        "value_layer": value_layer,
        "attention_mask": attention_mask,
    })
    return out
